# revision 46
# baseline (speedup 1.0000x reference)
"""GCN 2-layer kernel for trn2: host preprocessing + Bass kernel builder (v2).

Math (per GCNConv, PyG-style):
  out = D^-1/2 (A+I) D^-1/2 (X W) + b ;  layer1 -> relu -> layer2.

Device plan (8 cores, SPMD), per core:
  P1: h1' = dinv .* (x_shard @ W1)   (node/pos-sharded)    -> h1p_sb (SBUF)
      AG1 in 4 pieces (one per src window of 25 blocks)
  P3: per group of 20 dst blocks: 4 dma_gathers (one per src window),
      indicator matmuls accumulate into 5 PSUM bank tiles [128,512]
      (4 blocks per bank), self-loops seeded via diag(dinv) matmuls,
      finalize: dinv scale (DVE), relu+b1 (ACT), @W2, dinv scale -> h2p_sb
      AG2 pieces as windows complete.
  P5: same aggregation transposed ([CPAD, dst] psum) + b2 -> out_sT.
Host: transpose/unpermute rows, slice [:N0, :CLS].

Edge streams are layer-invariant (both layers gather by table position),
so idx/dl tables are shared between the two aggregation phases.
Chunks are 128 edges targeting a 64-wide dst window; windows march
uniformly across each 512-dst psum segment so chunk geometry is
core-invariant (required: the program is compiled once for all cores).
"""

from dataclasses import dataclass

import numpy as np

import concourse.bass as bass
import concourse.mybir as mybir
import concourse.tile as tile
from concourse import bacc

FP = mybir.dt.float32
BF = mybir.dt.bfloat16


@dataclass
class Cfg:
    N0: int = 100000     # real nodes
    W: int = 8           # cores
    SHARD: int = 12800   # nodes per core
    NB: int = 100        # blocks per core
    F: int = 512         # in features
    HID: int = 128
    CLS: int = 40
    CPAD: int = 64       # padded class width (stream/matmul width)
    TPAD: int = 128      # L2 table row elems (256B gather-elem rule)
    GB: int = 20         # dst blocks per psum group
    NG: int = 5          # groups per core
    QB: int = 25         # blocks per AG window (piece)
    NW: int = 4          # src windows
    WIN: int = 64        # chunk dst-window width
    SEG: int = 512       # dsts per psum bank tile
    NSEG: int = 5        # segments per group (GB*128/SEG)

    @property
    def NP(self):
        return self.W * self.SHARD          # 102400

    @property
    def WROWS(self):
        return self.W * self.QB * 128       # 25600 table rows per window

    @property
    def CROWS(self):
        return self.QB * 128                # 3200 rows per core per window


@dataclass
class Meta:
    kq: np.ndarray = None          # [NG, NW, NSEG] chunks per bucket-segment
    f0s: dict = None               # (g,w,t) -> np.ndarray of window starts
    bucket_off: np.ndarray = None  # [NG, NW] chunk-stream offset
    bucket_n: np.ndarray = None    # [NG, NW] chunks in bucket
    CT: int = 0                    # total chunks
    node_of_pos: np.ndarray = None  # [W, SHARD]
    debug: list = None             # per-core (idx_pad, dl_pad) for simcheck


def _serpentine(cfg, deg):
    """Degree-balanced node->position assignment (serpentine over blocks)."""
    W, SHARD, NB, NP, N0 = cfg.W, cfg.SHARD, cfg.NB, cfg.NP, cfg.N0
    nblocks = W * NB
    order = np.argsort(-deg[:N0], kind="stable")
    all_ids = np.concatenate([order, np.arange(N0, NP, dtype=np.int64)])
    r = np.arange(NP, dtype=np.int64)
    cyc = r % (2 * nblocks)
    blk = np.where(cyc < nblocks, cyc, 2 * nblocks - 1 - cyc)
    slot_ctr = r // (2 * nblocks) * 2 + (cyc >= nblocks).astype(np.int64)
    pos = (blk % W) * SHARD + (blk // W) * 128 + slot_ctr
    pos_of_node = np.empty(NP, dtype=np.int64)
    pos_of_node[all_ids] = pos
    node_of_pos = np.empty(NP, dtype=np.int64)
    node_of_pos[pos] = all_ids
    return pos_of_node, node_of_pos


def _window_starts(cfg, kq):
    """kq marching WIN-wide windows covering [0, SEG) (uniform fallback)."""
    if kq == 1:
        return np.zeros(1, dtype=np.int64)
    span = cfg.SEG - cfg.WIN
    return np.round(np.arange(kq) * span / (kq - 1)).astype(np.int64)


def _schedule_windows(cfg, ds_by_core, cap=126):
    """Capacity-driven shared window schedule for one (g,w,t) segment.

    ds_by_core: list of sorted dst-offset arrays (one per core).
    Returns f0 array such that greedy front-fill succeeds for every core:
    windows advance by at most WIN (coverage) and no faster than any
    core's edge supply allows (Hall's condition: edges below f0[j] must
    fit into the first j windows).
    """
    SEG, WIN = cfg.SEG, cfg.WIN
    top = SEG - WIN
    # max over cores of cumulative count below v, for v in 0..SEG
    cum = np.zeros(SEG + 1, dtype=np.int64)
    for ds in ds_by_core:
        c = np.searchsorted(ds, np.arange(SEG + 1))
        np.maximum(cum, c, out=cum)
    total = int(cum[SEG])
    f0 = [0]
    while not (f0[-1] >= top and cap * len(f0) >= total):
        j = len(f0)
        # furthest v with max-core cum(v) <= cap*j (Hall's condition)
        v = int(np.searchsorted(cum, cap * j, side="right") - 1)
        v = max(min(v & ~31, f0[-1] + WIN, top), f0[-1])  # 32-aligned psum
        f0.append(v)
        if len(f0) > 300:
            raise RuntimeError("window schedule runaway")
    return np.asarray(f0, dtype=np.int64)


def _assign_core(cfg, ds_sorted, seg_bounds, kq, f0s):
    """Greedy front-fill of sorted dst offsets into marching windows.

    Returns (per-key list of (lo, hi) chunk ranges, failing keys).
    """
    NGWT = cfg.NG * cfg.NW * cfg.NSEG
    ranges = [None] * NGWT
    fails = []
    kqf = kq.reshape(-1)
    for key in range(NGWT):
        lo, hi = seg_bounds[key], seg_bounds[key + 1]
        ds = ds_sorted[lo:hi]
        n = hi - lo
        k = int(kqf[key])
        f0 = f0s[(key // (cfg.NW * cfg.NSEG),
                  (key // cfg.NSEG) % cfg.NW, key % cfg.NSEG)]
        ptr = 0
        rr = []
        ok = True
        for j in range(k):
            if ptr < n and ds[ptr] < f0[j]:
                ok = False
                break
            end = int(np.searchsorted(ds, f0[j] + cfg.WIN, side="left"))
            take = max(min(end - ptr, 128), 0)
            rr.append((ptr, ptr + take))
            ptr += take
        if not ok or ptr < n:
            fails.append(key)
        else:
            ranges[key] = rr
    return ranges, fails


def preprocess(cfg: Cfg, x, edge_index, W1, b1, W2, b2):
    W, SHARD, NP, N0 = cfg.W, cfg.SHARD, cfg.NP, cfg.N0
    NB, GB, NG, QB, NW, WIN, SEG, NSEG = (cfg.NB, cfg.GB, cfg.NG, cfg.QB,
                                          cfg.NW, cfg.WIN, cfg.SEG, cfg.NSEG)
    x = np.asarray(x, dtype=np.float32)
    edge_index = np.asarray(edge_index)
    W1 = np.asarray(W1, np.float32)
    b1 = np.asarray(b1, np.float32)
    W2 = np.asarray(W2, np.float32)
    b2 = np.asarray(b2, np.float32)

    s = edge_index[0].astype(np.int64)
    d = edge_index[1].astype(np.int64)
    loops = np.arange(N0, dtype=np.int64)
    d_all = np.concatenate([d, loops])
    deg = np.bincount(d_all, minlength=NP).astype(np.float64)
    with np.errstate(divide="ignore"):
        dinv = np.where(deg > 0, 1.0 / np.sqrt(deg), 0.0).astype(np.float32)

    pos_of_node, node_of_pos = _serpentine(cfg, deg)
    dinv_pos = dinv[node_of_pos]  # [NP] dinv in position order

    # --- edge -> (core, key=(g,w,t), dseg, rel) ---
    pd = pos_of_node[d]
    ps = pos_of_node[s]
    core = pd // SHARD
    locd = pd % SHARD
    bd = locd // 128
    sd = locd % 128
    g = bd // GB
    r = bd % GB
    dgrp = r * 128 + sd
    t = dgrp // SEG
    dseg = dgrp % SEG
    cs = ps // SHARD
    locs = ps % SHARD
    bs = locs // 128
    ss = locs % 128
    w = cs // 2                          # window = core pair
    rel = (cs % 2) * SHARD + ss * NB + bs  # row within window (s-major)

    NGWT = NG * NW * NSEG
    key = (g * NW + w) * NSEG + t

    core_data = []
    counts = np.zeros((W, NGWT), dtype=np.int64)
    for c in range(W):
        m = core == c
        kc, dc, rc = key[m], dseg[m], rel[m]
        o = np.lexsort((dc, kc))
        kc, dc, rc = kc[o], dc[o], rc[o]
        bounds = np.searchsorted(kc, np.arange(NGWT + 1))
        counts[c] = bounds[1:] - bounds[:-1]
        core_data.append((dc, rc, bounds))

    # capacity-driven shared window schedules per (g, w, t)
    def seg_ds(kk):
        out = []
        for c in range(W):
            dc, rc, bounds = core_data[c]
            out.append(dc[bounds[kk]:bounds[kk + 1]])
        return out

    f0s = {}
    kq = np.zeros((NG, NW, NSEG), dtype=np.int64)
    caps = {}
    for gg in range(NG):
        for ww in range(NW):
            for tt in range(NSEG):
                kk = (gg * NW + ww) * NSEG + tt
                f0 = _schedule_windows(cfg, seg_ds(kk))
                f0s[(gg, ww, tt)] = f0
                kq[gg, ww, tt] = len(f0)
                caps[kk] = 120

    all_ranges = None
    for _try in range(40):
        all_ranges = []
        fail_set = set()
        for c in range(W):
            dc, rc, bounds = core_data[c]
            ranges, fails = _assign_core(cfg, dc, bounds, kq, f0s)
            all_ranges.append(ranges)
            fail_set.update(fails)
        if not fail_set:
            break
        for kk in fail_set:
            gg, ww, tt = (kk // (NW * NSEG), (kk // NSEG) % NW, kk % NSEG)
            caps[kk] -= 12
            f0 = _schedule_windows(cfg, seg_ds(kk), cap=caps[kk])
            f0s[(gg, ww, tt)] = f0
            kq[gg, ww, tt] = len(f0)
    else:
        raise RuntimeError("chunk assignment did not converge")

    CT = int(kq.sum())
    bucket_n = kq.sum(axis=2)                      # [NG, NW]
    bucket_off = np.zeros((NG, NW), dtype=np.int64)
    bucket_off.reshape(-1)[1:] = np.cumsum(bucket_n.reshape(-1))[:-1]

    # --- build idx / dl tables per core ---
    idx16_all, dl_all, debug_all = [], [], []
    for c in range(W):
        dc, rc, bounds = core_data[c]
        ranges = all_ranges[c]
        idx_pad = np.zeros(CT * 128, dtype=np.int64)
        dl_pad = np.full(CT * 128, -1.0, dtype=np.float32)
        ck_global = 0
        for gg in range(NG):
            for ww in range(NW):
                for tt in range(NSEG):
                    kk = (gg * NW + ww) * NSEG + tt
                    lo = bounds[kk]
                    f0 = f0s[(gg, ww, tt)]
                    for j, (alo, ahi) in enumerate(ranges[kk]):
                        nn = ahi - alo
                        base = ck_global * 128
                        if nn > 0:
                            rr = rc[lo + alo:lo + ahi]
                            dd = dc[lo + alo:lo + ahi] - f0[j]
                            o2 = np.argsort(rr, kind="stable")  # HBM locality
                            idx_pad[base:base + nn] = rr[o2]
                            dl_pad[base:base + nn] = dd[o2]
                        ck_global += 1
        assert ck_global == CT
        a = idx_pad.reshape(CT, 8, 16)
        wrapped = a.transpose(2, 0, 1).reshape(16, CT * 8)
        idx16_all.append(np.tile(wrapped, (8, 1)).astype(np.int16))
        dl_all.append(np.broadcast_to(
            dl_pad.reshape(CT, 128).T, (128, CT)).copy())
        debug_all.append((idx_pad.copy(), dl_pad.copy()))

    # --- per-core dense tensors ---
    import ml_dtypes
    bft = ml_dtypes.bfloat16
    per_core = []
    iota64 = np.ascontiguousarray(np.broadcast_to(
        np.arange(WIN, dtype=np.float32), (128, WIN))).astype(bft)
    sl = np.arange(128)
    w1k_h = np.ascontiguousarray(
        W1.reshape(4, 128, cfg.HID).transpose(1, 0, 2)).astype(bft)
    for c in range(W):
        ids = node_of_pos[c * SHARD:(c + 1) * SHARD]
        xs = np.where((ids < N0)[:, None], x[np.minimum(ids, N0 - 1)], 0.0)
        dpos = dinv_pos[c * SHARD:(c + 1) * SHARD].astype(np.float32)
        # xq[p, b, k, n] = x[node(b*128+n), k*128+p]
        xq = np.ascontiguousarray(
            xs.reshape(NB, 128, 4, 128).transpose(3, 0, 2, 1)).astype(bft)
        drt = np.ascontiguousarray(
            np.broadcast_to(dpos, (128, SHARD))).astype(bft)
        dmat = dpos.reshape(NB, 128)
        inp = {
            "xq": xq.reshape(128, NB * 4 * 128),
            "w1k": w1k_h.reshape(128, 4 * cfg.HID),
            "w2t": np.pad(W2, ((0, 0), (0, cfg.CPAD - cfg.CLS))).astype(bft),
            "b1col": b1.reshape(cfg.HID, 1).copy(),
            "b2rep": np.ascontiguousarray(np.broadcast_to(
                np.pad(b2, (0, cfg.CPAD - cfg.CLS))[:, None],
                (cfg.CPAD, 128))),
            "iota64": iota64,
            "dinv_pc": np.ascontiguousarray(dmat.T),   # [128, NB] fp32
            "drt": drt,                                 # [128, SHARD] bf16
            "ident": np.eye(128, dtype=np.float32).astype(bft),
            "idx16": idx16_all[c],                      # [128, CT*8] int16
            "dl": dl_all[c].astype(bft),                # [128, CT] bf16
        }
        per_core.append(inp)

    meta = Meta(kq=kq, f0s=f0s, bucket_off=bucket_off, bucket_n=bucket_n,
                CT=CT, node_of_pos=node_of_pos.reshape(W, SHARD),
                debug=debug_all)
    return per_core, meta, dinv


def postprocess(cfg: Cfg, outs, meta: Meta):
    """outs: list of [CPAD, SHARD] per core -> [N0, CLS] node order."""
    res = np.zeros((cfg.NP, cfg.CPAD), np.float32)
    for c in range(cfg.W):
        res[meta.node_of_pos[c]] = outs[c].T
    return res[:cfg.N0, :cfg.CLS]


def build(cfg: Cfg, meta: Meta):
    W, SHARD, NB, HID, CPAD, TPAD = (cfg.W, cfg.SHARD, cfg.NB, cfg.HID,
                                     cfg.CPAD, cfg.TPAD)
    GB, NG, QB, NW, WIN, SEG, NSEG = (cfg.GB, cfg.NG, cfg.QB, cfg.NW,
                                      cfg.WIN, cfg.SEG, cfg.NSEG)
    CT = meta.CT
    kq, f0s = meta.kq, meta.f0s
    bucket_off, bucket_n = meta.bucket_off, meta.bucket_n
    WROWS = cfg.WROWS
    AF = mybir.ActivationFunctionType

    nc = bacc.Bacc("TRN2", target_bir_lowering=False, debug=False,
                   num_devices=W, num_swdge_queues=4)

    xq = nc.dram_tensor("xq", [128, NB, 4, 128], BF, kind="ExternalInput")
    w1k = nc.dram_tensor("w1k", [128, 4, HID], BF, kind="ExternalInput")
    w2t = nc.dram_tensor("w2t", [HID, CPAD], BF, kind="ExternalInput")
    b1col = nc.dram_tensor("b1col", [HID, 1], FP, kind="ExternalInput")
    b2rep = nc.dram_tensor("b2rep", [CPAD, 128], FP, kind="ExternalInput")
    iota64 = nc.dram_tensor("iota64", [128, WIN], BF, kind="ExternalInput")
    dinv_pc = nc.dram_tensor("dinv_pc", [128, NB], FP, kind="ExternalInput")
    drt = nc.dram_tensor("drt", [128, SHARD], BF, kind="ExternalInput")
    ident = nc.dram_tensor("ident", [128, 128], BF, kind="ExternalInput")
    idx16 = nc.dram_tensor("idx16", [128, CT * 8], mybir.dt.int16,
                           kind="ExternalInput")
    dl = nc.dram_tensor("dl", [128, CT], BF, kind="ExternalInput")
    out_sT = nc.dram_tensor("out_sT", [CPAD, SHARD], FP, kind="ExternalOutput")

    NP = cfg.NP
    ag1_in = nc.dram_tensor("ag1_in", [128, NB, HID], BF)
    ag1_out = nc.dram_tensor("ag1_out", [NP, HID], BF, addr_space="Shared")
    ag2_in = nc.dram_tensor("ag2_in", [128, NB, TPAD], BF)
    ag2_out = nc.dram_tensor("ag2_out", [NP, TPAD], BF, addr_space="Shared")

    qctr = [0]

    def next_q():
        qctr[0] = (qctr[0] + 1) % 4
        return qctr[0]

    nmax = int(bucket_n.max())

    with tile.TileContext(nc) as tc:
        with (
            tc.tile_pool(name="const", bufs=1) as cpool,
            tc.tile_pool(name="xqp", bufs=4) as xqpool,
            tc.tile_pool(name="gath", bufs=2) as gpool,
            tc.tile_pool(name="indp", bufs=2) as ipool,
            tc.tile_pool(name="dgp", bufs=2) as dgpool,
            tc.tile_pool(name="fin", bufs=3) as fpool,
            tc.tile_pool(name="outp", bufs=1) as opool,
            tc.tile_pool(name="ps", bufs=1, space="PSUM") as pspool,
        ):
            # ---- constants ----
            w1k_t = cpool.tile([128, 4, HID], BF)
            nc.sync.dma_start(out=w1k_t[:, :, :], in_=w1k[:, :, :])
            w2_t = cpool.tile([HID, CPAD], BF)
            nc.sync.dma_start(out=w2_t[:, :], in_=w2t[:, :])
            b1_t = cpool.tile([HID, 1], FP)
            nc.sync.dma_start(out=b1_t[:, :], in_=b1col[:, :])
            b2_t = cpool.tile([CPAD, 128], FP)
            nc.sync.dma_start(out=b2_t[:, :], in_=b2rep[:, :])
            id_t = cpool.tile([128, 128], BF)
            nc.sync.dma_start(out=id_t[:, :], in_=ident[:, :])
            iota_t = cpool.tile([128, WIN], BF)
            nc.sync.dma_start(out=iota_t[:, :], in_=iota64[:, :])
            dpc_t = cpool.tile([128, NB], FP)
            nc.sync.dma_start(out=dpc_t[:, :], in_=dinv_pc[:, :])
            idx_t = cpool.tile([128, CT * 8], mybir.dt.int16)
            nc.sync.dma_start(out=idx_t[:, :], in_=idx16[:, :])
            dl_t = cpool.tile([128, CT], BF)
            nc.sync.dma_start(out=dl_t[:, :], in_=dl[:, :])
            h1p_sb = cpool.tile([128, NB, HID], BF)
            h2p_sb = cpool.tile([128, NB, CPAD], BF)

            # ---- P1: h1' = dinv .* (x @ W1) ----
            for b in range(NB):
                xq_b = xqpool.tile([128, 4, 128], BF, tag="xq")
                nc.sync.dma_start(out=xq_b[:, :, :], in_=xq[:, b, :, :])
                psh = pspool.tile([128, HID], FP, tag="psml", bufs=2)
                for k in range(4):
                    nc.tensor.matmul(out=psh[:, :], lhsT=xq_b[:, k, :],
                                     rhs=w1k_t[:, k, :],
                                     start=(k == 0), stop=(k == 3))
                nc.scalar.activation(out=h1p_sb[:, b, :], in_=psh[:, :],
                                     func=AF.Copy, scale=dpc_t[:, b:b + 1])
            nc.sync.dma_start(out=ag1_in[:, :, :], in_=h1p_sb[:, :, :])
            nc.gpsimd.collective_compute(
                "AllGather", mybir.AluOpType.bypass,
                replica_groups=[list(range(W))],
                ins=[ag1_in[:, :, :]], outs=[ag1_out[:, :]],
            )

            # ---- aggregation phases ----
            def agg_phase(layer):
                for g in range(NG):
                    drt_g = dgpool.tile([128, GB * 128], BF, tag="drt")
                    nc.sync.dma_start(
                        out=drt_g[:, :],
                        in_=drt[:, g * GB * 128:(g + 1) * GB * 128])
                    aggs = [pspool.tile([128, SEG], FP, tag="agg", bufs=NSEG,
                                        name=f"agg_l{layer}_g{g}_{t}")
                            for t in range(NSEG)]
                    outg = None
                    if layer == 2:
                        outg = opool.tile([CPAD, GB * 128], FP, tag="outg")
                    # seeds (self-loops): start accumulation groups
                    for r in range(GB):
                        b = g * GB + r
                        tt, fo = r // 4, (r % 4) * 128
                        # start=True resets the whole PSUM bank -> only the
                        # first seed per bank tile may use it.
                        st = (r % 4 == 0)
                        if layer == 1:
                            nc.tensor.matmul(
                                out=aggs[tt][:, fo:fo + 128],
                                lhsT=h1p_sb[:, b, :],
                                rhs=id_t[:, :],
                                start=st, stop=False, skip_group_check=True)
                        else:
                            nc.tensor.matmul(
                                out=aggs[tt][0:CPAD, fo:fo + 128],
                                lhsT=h2p_sb[:, b, :],
                                rhs=id_t[:, :],
                                start=st, stop=False, skip_group_check=True)

                    # gathers (split in halves) + indicators + chunk matmuls
                    hmax = (nmax + 1) // 2
                    for w in range(NW):
                        o = int(bucket_off[g, w])
                        n = int(bucket_n[g, w])
                        n1 = n // 2
                        src = ag1_out if layer == 1 else ag2_out
                        src = src[w * cfg.WROWS:(w + 1) * cfg.WROWS, :]
                        halves = []
                        for (ho, hn) in ((o, n1), (o + n1, n - n1)):
                            gb = gpool.tile([128, hmax, 128], BF, tag="gb",
                                            bufs=4, name=f"gb{layer}{g}{w}")
                            nc.gpsimd.dma_gather(
                                gb[:, 0:hn, :], src,
                                idx_t[:, ho * 8:(ho + hn) * 8],
                                hn * 128, hn * 128, 128,
                                single_packet=False,
                                queue_num=next_q(),
                            )
                            ind = ipool.tile([128, hmax, WIN], BF, tag="ind",
                                             bufs=4, name=f"in{layer}{g}{w}")
                            nc.vector.tensor_tensor(
                                out=ind[:, 0:hn, :],
                                in0=dl_t[:, ho:ho + hn].to_broadcast(
                                    [128, hn, WIN]),
                                in1=iota_t[:, None, :].to_broadcast(
                                    [128, hn, WIN]),
                                op=mybir.AluOpType.is_equal,
                            )
                            halves.append((gb, ind))
                        i = 0
                        for tt in range(NSEG):
                            nk = int(kq[g, w, tt])
                            f0 = f0s[(g, w, tt)]
                            for j in range(nk):
                                fo = int(f0[j])
                                stop = (w == NW - 1 and j == nk - 1)
                                gb, ind = halves[0 if i < n1 else 1]
                                i2 = i if i < n1 else i - n1
                                if layer == 1:
                                    nc.tensor.matmul(
                                        out=aggs[tt][:, fo:fo + WIN],
                                        lhsT=gb[:, i2, :],
                                        rhs=ind[:, i2, :],
                                        start=False, stop=stop,
                                        skip_group_check=True)
                                else:
                                    nc.tensor.matmul(
                                        out=aggs[tt][0:CPAD, fo:fo + WIN],
                                        lhsT=gb[:, i2, 0:CPAD],
                                        rhs=ind[:, i2, :],
                                        start=False, stop=stop,
                                        skip_group_check=True)
                                i += 1
                    # finalize blocks
                    for r in range(GB):
                        b = g * GB + r
                        tt, fo = r // 4, (r % 4) * 128
                        if layer == 1:
                            t1 = fpool.tile([128, 128], FP, tag="t1")
                            nc.vector.tensor_tensor(
                                out=t1[:, :], in0=aggs[tt][:, fo:fo + 128],
                                in1=drt_g[:, r * 128:(r + 1) * 128],
                                op=mybir.AluOpType.mult)
                            r1 = fpool.tile([128, 128], BF, tag="r1")
                            nc.scalar.activation(out=r1[:, :], in_=t1[:, :],
                                                 func=AF.Relu,
                                                 bias=b1_t[:, :1])
                            ps2 = pspool.tile([128, CPAD], FP, tag="psml",
                                              bufs=2)
                            nc.tensor.matmul(out=ps2[:, :], lhsT=r1[:, :],
                                             rhs=w2_t[:, :],
                                             start=True, stop=True)
                            nc.scalar.activation(out=h2p_sb[:, b, :],
                                                 in_=ps2[:, :],
                                                 func=AF.Copy,
                                                 scale=dpc_t[:, b:b + 1])
                        else:
                            t3 = fpool.tile([CPAD, 128], FP, tag="t3")
                            nc.vector.tensor_tensor(
                                out=t3[:, :],
                                in0=aggs[tt][0:CPAD, fo:fo + 128],
                                in1=drt_g[0:CPAD, r * 128:(r + 1) * 128],
                                op=mybir.AluOpType.mult)
                            nc.vector.tensor_tensor(
                                out=outg[0:CPAD, r * 128:(r + 1) * 128],
                                in0=t3[:, :], in1=b2_t[:, :],
                                op=mybir.AluOpType.add)
                    # group epilogue
                    if layer == 2:
                        nc.sync.dma_start(
                            out=out_sT[:, g * GB * 128:(g + 1) * GB * 128],
                            in_=outg[:, :])

            agg_phase(1)
            nc.sync.dma_start(out=ag2_in[:, :, 0:CPAD], in_=h2p_sb[:, :, :])
            nc.gpsimd.collective_compute(
                "AllGather", mybir.AluOpType.bypass,
                replica_groups=[list(range(W))],
                ins=[ag2_in[:, :, :]], outs=[ag2_out[:, :]],
            )
            agg_phase(2)

    nc.compile()
    return nc


# ======================================================================
# kernel() entry point
# ======================================================================
import os as _os


LAST_EXEC_NS = None
LAST_RES = None


def kernel(x, edge_index, W1, b1, W2, b2):
    """Full-input GCN kernel: shards across 8 NeuronCores internally."""
    global LAST_EXEC_NS, LAST_RES
    import numpy as _np

    trace = bool(int(_os.environ.get("GCN_TRACE", "0")))
    if trace:
        # Optional NTFF profiling shim (axon): non-fatal if unavailable.
        try:
            import sys as _sys
            import types as _types
            from trn_agent_boot.trn_boot import _ntff_profile_via_ctypes
            if "antenv.axon_hooks" not in _sys.modules:
                _hook = _ntff_profile_via_ctypes("/opt/axon/libaxon_pjrt.so")
                _m = _types.ModuleType("antenv.axon_hooks")
                _m.get_axon_ntff_profile_hook = lambda: _hook
                _m.set_axon_ntff_profile_hook = lambda h: None
                _sys.modules["antenv.axon_hooks"] = _m
        except Exception:
            trace = False

    from concourse.bass_utils import run_bass_kernel_spmd

    cfg = Cfg()
    per_core, meta, _ = preprocess(cfg, x, edge_index, W1, b1, W2, b2)
    nc = build(cfg, meta)
    res = run_bass_kernel_spmd(
        nc, per_core, core_ids=list(range(cfg.W)), trace=trace,
    )
    LAST_EXEC_NS = res.exec_time_ns
    LAST_RES = res
    outs = [res.results[c]["out_sT"] for c in range(cfg.W)]
    return _np.ascontiguousarray(postprocess(cfg, outs, meta).astype(_np.float32))


# revision 47
# speedup vs baseline: 1.2390x; 1.2390x over previous
"""GCN 2-layer kernel for trn2: host preprocessing + Bass kernel builder (v2).

Math (per GCNConv, PyG-style):
  out = D^-1/2 (A+I) D^-1/2 (X W) + b ;  layer1 -> relu -> layer2.

Device plan (8 cores, SPMD), per core:
  P1: h1' = dinv .* (x_shard @ W1)   (node/pos-sharded)    -> h1p_sb (SBUF)
      AG1 in 4 pieces (one per src window of 25 blocks)
  P3: per group of 20 dst blocks: 4 dma_gathers (one per src window),
      indicator matmuls accumulate into 5 PSUM bank tiles [128,512]
      (4 blocks per bank), self-loops seeded via diag(dinv) matmuls,
      finalize: dinv scale (DVE), relu+b1 (ACT), @W2, dinv scale -> h2p_sb
      AG2 pieces as windows complete.
  P5: same aggregation transposed ([CPAD, dst] psum) + b2 -> out_sT.
Host: transpose/unpermute rows, slice [:N0, :CLS].

Edge streams are layer-invariant (both layers gather by table position),
so idx/dl tables are shared between the two aggregation phases.
Chunks are 128 edges targeting a 64-wide dst window; windows march
uniformly across each 512-dst psum segment so chunk geometry is
core-invariant (required: the program is compiled once for all cores).
"""

from dataclasses import dataclass

import numpy as np

import concourse.bass as bass
import concourse.mybir as mybir
import concourse.tile as tile
from concourse import bacc

FP = mybir.dt.float32
BF = mybir.dt.bfloat16


@dataclass
class Cfg:
    N0: int = 100000     # real nodes
    W: int = 8           # cores
    SHARD: int = 12800   # nodes per core
    NB: int = 100        # blocks per core
    F: int = 512         # in features
    HID: int = 128
    CLS: int = 40
    CPAD: int = 64       # padded class width (stream/matmul width)
    TPAD: int = 128      # L2 table row elems (256B gather-elem rule)
    GB: int = 20         # dst blocks per psum group
    NG: int = 5          # groups per core
    QB: int = 25         # blocks per AG window (piece)
    NW: int = 4          # src windows
    WIN: int = 64        # chunk dst-window width
    SEG: int = 512       # dsts per psum bank tile
    NSEG: int = 5        # segments per group (GB*128/SEG)

    @property
    def NP(self):
        return self.W * self.SHARD          # 102400

    @property
    def WROWS(self):
        return self.W * self.QB * 128       # 25600 table rows per window

    @property
    def CROWS(self):
        return self.QB * 128                # 3200 rows per core per window


@dataclass
class Meta:
    kq: np.ndarray = None          # [NG, NW, NSEG] chunks per bucket-segment
    f0s: dict = None               # (g,w,t) -> np.ndarray of window starts
    bucket_off: np.ndarray = None  # [NG, NW] chunk-stream offset
    bucket_n: np.ndarray = None    # [NG, NW] chunks in bucket
    CT: int = 0                    # total chunks
    node_of_pos: np.ndarray = None  # [W, SHARD]
    debug: list = None             # per-core (idx_pad, dl_pad) for simcheck


def _serpentine(cfg, deg):
    """Degree-balanced node->position assignment (serpentine over blocks)."""
    W, SHARD, NB, NP, N0 = cfg.W, cfg.SHARD, cfg.NB, cfg.NP, cfg.N0
    nblocks = W * NB
    order = np.argsort(-deg[:N0], kind="stable")
    all_ids = np.concatenate([order, np.arange(N0, NP, dtype=np.int64)])
    r = np.arange(NP, dtype=np.int64)
    cyc = r % (2 * nblocks)
    blk = np.where(cyc < nblocks, cyc, 2 * nblocks - 1 - cyc)
    slot_ctr = r // (2 * nblocks) * 2 + (cyc >= nblocks).astype(np.int64)
    pos = (blk % W) * SHARD + (blk // W) * 128 + slot_ctr
    pos_of_node = np.empty(NP, dtype=np.int64)
    pos_of_node[all_ids] = pos
    node_of_pos = np.empty(NP, dtype=np.int64)
    node_of_pos[pos] = all_ids
    return pos_of_node, node_of_pos


def _window_starts(cfg, kq):
    """kq marching WIN-wide windows covering [0, SEG) (uniform fallback)."""
    if kq == 1:
        return np.zeros(1, dtype=np.int64)
    span = cfg.SEG - cfg.WIN
    return np.round(np.arange(kq) * span / (kq - 1)).astype(np.int64)


def _schedule_windows(cfg, ds_by_core, cap=126):
    """Capacity-driven shared window schedule for one (g,w,t) segment.

    ds_by_core: list of sorted dst-offset arrays (one per core).
    Returns f0 array such that greedy front-fill succeeds for every core:
    windows advance by at most WIN (coverage) and no faster than any
    core's edge supply allows (Hall's condition: edges below f0[j] must
    fit into the first j windows).
    """
    SEG, WIN = cfg.SEG, cfg.WIN
    top = SEG - WIN
    # max over cores of cumulative count below v, for v in 0..SEG
    cum = np.zeros(SEG + 1, dtype=np.int64)
    for ds in ds_by_core:
        c = np.searchsorted(ds, np.arange(SEG + 1))
        np.maximum(cum, c, out=cum)
    total = int(cum[SEG])
    f0 = [0]
    while not (f0[-1] >= top and cap * len(f0) >= total):
        j = len(f0)
        # furthest v with max-core cum(v) <= cap*j (Hall's condition)
        v = int(np.searchsorted(cum, cap * j, side="right") - 1)
        v = max(min(v & ~31, f0[-1] + WIN, top), f0[-1])  # 32-aligned psum
        f0.append(v)
        if len(f0) > 300:
            raise RuntimeError("window schedule runaway")
    return np.asarray(f0, dtype=np.int64)


def _assign_core(cfg, ds_sorted, seg_bounds, kq, f0s):
    """Greedy front-fill of sorted dst offsets into marching windows.

    Returns (per-key list of (lo, hi) chunk ranges, failing keys).
    """
    NGWT = cfg.NG * cfg.NW * cfg.NSEG
    ranges = [None] * NGWT
    fails = []
    kqf = kq.reshape(-1)
    for key in range(NGWT):
        lo, hi = seg_bounds[key], seg_bounds[key + 1]
        ds = ds_sorted[lo:hi]
        n = hi - lo
        k = int(kqf[key])
        f0 = f0s[(key // (cfg.NW * cfg.NSEG),
                  (key // cfg.NSEG) % cfg.NW, key % cfg.NSEG)]
        ptr = 0
        rr = []
        ok = True
        for j in range(k):
            if ptr < n and ds[ptr] < f0[j]:
                ok = False
                break
            end = int(np.searchsorted(ds, f0[j] + cfg.WIN, side="left"))
            take = max(min(end - ptr, 128), 0)
            rr.append((ptr, ptr + take))
            ptr += take
        if not ok or ptr < n:
            fails.append(key)
        else:
            ranges[key] = rr
    return ranges, fails


def preprocess(cfg: Cfg, x, edge_index, W1, b1, W2, b2):
    W, SHARD, NP, N0 = cfg.W, cfg.SHARD, cfg.NP, cfg.N0
    NB, GB, NG, QB, NW, WIN, SEG, NSEG = (cfg.NB, cfg.GB, cfg.NG, cfg.QB,
                                          cfg.NW, cfg.WIN, cfg.SEG, cfg.NSEG)
    x = np.asarray(x, dtype=np.float32)
    edge_index = np.asarray(edge_index)
    W1 = np.asarray(W1, np.float32)
    b1 = np.asarray(b1, np.float32)
    W2 = np.asarray(W2, np.float32)
    b2 = np.asarray(b2, np.float32)

    s = edge_index[0].astype(np.int64)
    d = edge_index[1].astype(np.int64)
    loops = np.arange(N0, dtype=np.int64)
    d_all = np.concatenate([d, loops])
    deg = np.bincount(d_all, minlength=NP).astype(np.float64)
    with np.errstate(divide="ignore"):
        dinv = np.where(deg > 0, 1.0 / np.sqrt(deg), 0.0).astype(np.float32)

    pos_of_node, node_of_pos = _serpentine(cfg, deg)
    dinv_pos = dinv[node_of_pos]  # [NP] dinv in position order

    # --- edge -> (core, key=(g,w,t), dseg, rel) ---
    pd = pos_of_node[d]
    ps = pos_of_node[s]
    core = pd // SHARD
    locd = pd % SHARD
    bd = locd // 128
    sd = locd % 128
    g = bd // GB
    r = bd % GB
    dgrp = r * 128 + sd
    t = dgrp // SEG
    dseg = dgrp % SEG
    cs = ps // SHARD
    locs = ps % SHARD
    bs = locs // 128
    ss = locs % 128
    w = cs // 2                          # window = core pair
    rel = (cs % 2) * SHARD + ss * NB + bs  # row within window (s-major)

    NGWT = NG * NW * NSEG
    key = (g * NW + w) * NSEG + t

    core_data = []
    counts = np.zeros((W, NGWT), dtype=np.int64)
    for c in range(W):
        m = core == c
        kc, dc, rc = key[m], dseg[m], rel[m]
        o = np.lexsort((dc, kc))
        kc, dc, rc = kc[o], dc[o], rc[o]
        bounds = np.searchsorted(kc, np.arange(NGWT + 1))
        counts[c] = bounds[1:] - bounds[:-1]
        core_data.append((dc, rc, bounds))

    # capacity-driven shared window schedules per (g, w, t)
    def seg_ds(kk):
        out = []
        for c in range(W):
            dc, rc, bounds = core_data[c]
            out.append(dc[bounds[kk]:bounds[kk + 1]])
        return out

    f0s = {}
    kq = np.zeros((NG, NW, NSEG), dtype=np.int64)
    caps = {}
    for gg in range(NG):
        for ww in range(NW):
            for tt in range(NSEG):
                kk = (gg * NW + ww) * NSEG + tt
                f0 = _schedule_windows(cfg, seg_ds(kk))
                f0s[(gg, ww, tt)] = f0
                kq[gg, ww, tt] = len(f0)
                caps[kk] = 120

    all_ranges = None
    for _try in range(40):
        all_ranges = []
        fail_set = set()
        for c in range(W):
            dc, rc, bounds = core_data[c]
            ranges, fails = _assign_core(cfg, dc, bounds, kq, f0s)
            all_ranges.append(ranges)
            fail_set.update(fails)
        if not fail_set:
            break
        for kk in fail_set:
            gg, ww, tt = (kk // (NW * NSEG), (kk // NSEG) % NW, kk % NSEG)
            caps[kk] -= 12
            f0 = _schedule_windows(cfg, seg_ds(kk), cap=caps[kk])
            f0s[(gg, ww, tt)] = f0
            kq[gg, ww, tt] = len(f0)
    else:
        raise RuntimeError("chunk assignment did not converge")

    CT = int(kq.sum())
    bucket_n = kq.sum(axis=2)                      # [NG, NW]
    bucket_off = np.zeros((NG, NW), dtype=np.int64)
    bucket_off.reshape(-1)[1:] = np.cumsum(bucket_n.reshape(-1))[:-1]

    # --- build idx / dl tables per core ---
    idx16_all, dl_all, debug_all = [], [], []
    for c in range(W):
        dc, rc, bounds = core_data[c]
        ranges = all_ranges[c]
        idx_pad = np.zeros(CT * 128, dtype=np.int64)
        dl_pad = np.full(CT * 128, -1.0, dtype=np.float32)
        ck_global = 0
        for gg in range(NG):
            for ww in range(NW):
                for tt in range(NSEG):
                    kk = (gg * NW + ww) * NSEG + tt
                    lo = bounds[kk]
                    f0 = f0s[(gg, ww, tt)]
                    for j, (alo, ahi) in enumerate(ranges[kk]):
                        nn = ahi - alo
                        base = ck_global * 128
                        if nn > 0:
                            rr = rc[lo + alo:lo + ahi]
                            dd = dc[lo + alo:lo + ahi] - f0[j]
                            o2 = np.argsort(rr, kind="stable")  # HBM locality
                            idx_pad[base:base + nn] = rr[o2]
                            dl_pad[base:base + nn] = dd[o2]
                        ck_global += 1
        assert ck_global == CT
        a = idx_pad.reshape(CT, 8, 16)
        wrapped = a.transpose(2, 0, 1).reshape(16, CT * 8)
        idx16_all.append(np.tile(wrapped, (8, 1)).astype(np.int16))
        dl_all.append(np.broadcast_to(
            dl_pad.reshape(CT, 128).T, (128, CT)).copy())
        debug_all.append((idx_pad.copy(), dl_pad.copy()))

    # --- per-core dense tensors ---
    import ml_dtypes
    bft = ml_dtypes.bfloat16
    per_core = []
    iota64 = np.ascontiguousarray(np.broadcast_to(
        np.arange(WIN, dtype=np.float32), (128, WIN))).astype(bft)
    sl = np.arange(128)
    w1k_h = np.ascontiguousarray(
        W1.reshape(4, 128, cfg.HID).transpose(1, 0, 2)).astype(bft)
    for c in range(W):
        ids = node_of_pos[c * SHARD:(c + 1) * SHARD]
        xs = np.where((ids < N0)[:, None], x[np.minimum(ids, N0 - 1)], 0.0)
        dpos = dinv_pos[c * SHARD:(c + 1) * SHARD].astype(np.float32)
        # xq[p, b, k, n] = x[node(b*128+n), k*128+p]
        xq = np.ascontiguousarray(
            xs.reshape(NB, 128, 4, 128).transpose(3, 0, 2, 1)).astype(bft)
        drt = np.ascontiguousarray(
            np.broadcast_to(dpos, (128, SHARD))).astype(bft)
        dmat = dpos.reshape(NB, 128)
        inp = {
            "xq": xq.reshape(128, NB * 4 * 128),
            "w1k": w1k_h.reshape(128, 4 * cfg.HID),
            "w2t": np.pad(W2, ((0, 0), (0, cfg.CPAD - cfg.CLS))).astype(bft),
            "b1col": b1.reshape(cfg.HID, 1).copy(),
            "b2rep": np.ascontiguousarray(np.broadcast_to(
                np.pad(b2, (0, cfg.CPAD - cfg.CLS))[:, None],
                (cfg.CPAD, 128))),
            "iota64": iota64,
            "dinv_pc": np.ascontiguousarray(dmat.T),   # [128, NB] fp32
            "drt": drt,                                 # [128, SHARD] bf16
            "ident": np.eye(128, dtype=np.float32).astype(bft),
            "idx16": idx16_all[c],                      # [128, CT*8] int16
            "dl": dl_all[c].astype(bft),                # [128, CT] bf16
        }
        per_core.append(inp)

    meta = Meta(kq=kq, f0s=f0s, bucket_off=bucket_off, bucket_n=bucket_n,
                CT=CT, node_of_pos=node_of_pos.reshape(W, SHARD),
                debug=debug_all)
    return per_core, meta, dinv


def postprocess(cfg: Cfg, outs, meta: Meta):
    """outs: list of [CPAD, SHARD] per core -> [N0, CLS] node order."""
    res = np.zeros((cfg.NP, cfg.CPAD), np.float32)
    for c in range(cfg.W):
        res[meta.node_of_pos[c]] = outs[c].T
    return res[:cfg.N0, :cfg.CLS]


def build(cfg: Cfg, meta: Meta):
    W, SHARD, NB, HID, CPAD, TPAD = (cfg.W, cfg.SHARD, cfg.NB, cfg.HID,
                                     cfg.CPAD, cfg.TPAD)
    GB, NG, QB, NW, WIN, SEG, NSEG = (cfg.GB, cfg.NG, cfg.QB, cfg.NW,
                                      cfg.WIN, cfg.SEG, cfg.NSEG)
    CT = meta.CT
    kq, f0s = meta.kq, meta.f0s
    bucket_off, bucket_n = meta.bucket_off, meta.bucket_n
    WROWS = cfg.WROWS
    AF = mybir.ActivationFunctionType

    nc = bacc.Bacc("TRN2", target_bir_lowering=False, debug=False,
                   num_devices=W, num_swdge_queues=4)

    xq = nc.dram_tensor("xq", [128, NB, 4, 128], BF, kind="ExternalInput")
    w1k = nc.dram_tensor("w1k", [128, 4, HID], BF, kind="ExternalInput")
    w2t = nc.dram_tensor("w2t", [HID, CPAD], BF, kind="ExternalInput")
    b1col = nc.dram_tensor("b1col", [HID, 1], FP, kind="ExternalInput")
    b2rep = nc.dram_tensor("b2rep", [CPAD, 128], FP, kind="ExternalInput")
    iota64 = nc.dram_tensor("iota64", [128, WIN], BF, kind="ExternalInput")
    dinv_pc = nc.dram_tensor("dinv_pc", [128, NB], FP, kind="ExternalInput")
    drt = nc.dram_tensor("drt", [128, SHARD], BF, kind="ExternalInput")
    ident = nc.dram_tensor("ident", [128, 128], BF, kind="ExternalInput")
    idx16 = nc.dram_tensor("idx16", [128, CT * 8], mybir.dt.int16,
                           kind="ExternalInput")
    dl = nc.dram_tensor("dl", [128, CT], BF, kind="ExternalInput")
    out_sT = nc.dram_tensor("out_sT", [CPAD, SHARD], FP, kind="ExternalOutput")

    NP = cfg.NP
    ag1_in = nc.dram_tensor("ag1_in", [128, NB, HID], BF)
    ag1_out = nc.dram_tensor("ag1_out", [NP, HID], BF, addr_space="Shared")
    ag2_in = nc.dram_tensor("ag2_in", [128, NB, TPAD], BF)
    ag2_out = nc.dram_tensor("ag2_out", [NP, TPAD], BF, addr_space="Shared")

    qctr = [0]

    def next_q():
        qctr[0] = (qctr[0] + 1) % 4
        return qctr[0]

    nmax = int(bucket_n.max())

    with tile.TileContext(nc) as tc:
        with (
            tc.tile_pool(name="const", bufs=1) as cpool,
            tc.tile_pool(name="xqp", bufs=4) as xqpool,
            tc.tile_pool(name="gath", bufs=2) as gpool,
            tc.tile_pool(name="indp", bufs=2) as ipool,
            tc.tile_pool(name="dgp", bufs=2) as dgpool,
            tc.tile_pool(name="fin", bufs=3) as fpool,
            tc.tile_pool(name="outp", bufs=1) as opool,
            tc.tile_pool(name="ps", bufs=1, space="PSUM") as pspool,
        ):
            # ---- constants ----
            w1k_t = cpool.tile([128, 4, HID], BF)
            nc.sync.dma_start(out=w1k_t[:, :, :], in_=w1k[:, :, :])
            w2_t = cpool.tile([HID, CPAD], BF)
            nc.sync.dma_start(out=w2_t[:, :], in_=w2t[:, :])
            b1_t = cpool.tile([HID, 1], FP)
            nc.sync.dma_start(out=b1_t[:, :], in_=b1col[:, :])
            b2_t = cpool.tile([CPAD, 128], FP)
            nc.sync.dma_start(out=b2_t[:, :], in_=b2rep[:, :])
            id_t = cpool.tile([128, 128], BF)
            nc.sync.dma_start(out=id_t[:, :], in_=ident[:, :])
            iota_t = cpool.tile([128, WIN], BF)
            nc.sync.dma_start(out=iota_t[:, :], in_=iota64[:, :])
            dpc_t = cpool.tile([128, NB], FP)
            nc.sync.dma_start(out=dpc_t[:, :], in_=dinv_pc[:, :])
            idx_t = cpool.tile([128, CT * 8], mybir.dt.int16)
            nc.sync.dma_start(out=idx_t[:, :], in_=idx16[:, :])
            dl_t = cpool.tile([128, CT], BF)
            nc.sync.dma_start(out=dl_t[:, :], in_=dl[:, :])
            h1p_sb = cpool.tile([128, NB, HID], BF)
            h2p_sb = cpool.tile([128, NB, CPAD], BF)

            # ---- P1: h1' = dinv .* (x @ W1) ----
            for b in range(NB):
                xq_b = xqpool.tile([128, 4, 128], BF, tag="xq")
                nc.sync.dma_start(out=xq_b[:, :, :], in_=xq[:, b, :, :])
                psh = pspool.tile([128, HID], FP, tag="psml", bufs=2)
                for k in range(4):
                    nc.tensor.matmul(out=psh[:, :], lhsT=xq_b[:, k, :],
                                     rhs=w1k_t[:, k, :],
                                     start=(k == 0), stop=(k == 3))
                nc.scalar.activation(out=h1p_sb[:, b, :], in_=psh[:, :],
                                     func=AF.Copy, scale=dpc_t[:, b:b + 1])
            nc.sync.dma_start(out=ag1_in[:, :, :], in_=h1p_sb[:, :, :])
            nc.gpsimd.collective_compute(
                "AllGather", mybir.AluOpType.bypass,
                replica_groups=[list(range(W))],
                ins=[ag1_in[:, :, :]], outs=[ag1_out[:, :]],
            )

            # ---- aggregation phases ----
            def agg_phase(layer):
                for g in range(NG):
                    drt_g = dgpool.tile([128, GB * 128], BF, tag="drt")
                    nc.sync.dma_start(
                        out=drt_g[:, :],
                        in_=drt[:, g * GB * 128:(g + 1) * GB * 128])
                    aggs = [pspool.tile([128, SEG], FP, tag="agg", bufs=NSEG,
                                        name=f"agg_l{layer}_g{g}_{t}")
                            for t in range(NSEG)]
                    outg = None
                    if layer == 2:
                        outg = opool.tile([CPAD, GB * 128], FP, tag="outg")
                    # seeds (self-loops): start accumulation groups
                    for r in range(GB):
                        b = g * GB + r
                        tt, fo = r // 4, (r % 4) * 128
                        # start=True resets the whole PSUM bank -> only the
                        # first seed per bank tile may use it.
                        st = (r % 4 == 0)
                        if layer == 1:
                            nc.tensor.matmul(
                                out=aggs[tt][:, fo:fo + 128],
                                lhsT=h1p_sb[:, b, :],
                                rhs=id_t[:, :],
                                start=st, stop=False, skip_group_check=True)
                        else:
                            nc.tensor.matmul(
                                out=aggs[tt][0:CPAD, fo:fo + 128],
                                lhsT=h2p_sb[:, b, :],
                                rhs=id_t[:, :],
                                start=st, stop=False, skip_group_check=True)

                    # gathers (split in SUB parts) + indicators + chunk mms
                    SUB = 4
                    hmax = (nmax + SUB - 1) // SUB
                    for w in range(NW):
                        o = int(bucket_off[g, w])
                        n = int(bucket_n[g, w])
                        src = ag1_out if layer == 1 else ag2_out
                        src = src[w * cfg.WROWS:(w + 1) * cfg.WROWS, :]
                        bnds = [o + (n * q) // SUB for q in range(SUB + 1)]
                        parts = []
                        for q in range(SUB):
                            ho, hn = bnds[q], bnds[q + 1] - bnds[q]
                            gb = gpool.tile([128, hmax, 128], BF, tag="gb",
                                            bufs=6, name=f"gb{layer}{g}{w}")
                            nc.gpsimd.dma_gather(
                                gb[:, 0:hn, :], src,
                                idx_t[:, ho * 8:(ho + hn) * 8],
                                hn * 128, hn * 128, 128,
                                single_packet=False,
                                queue_num=next_q(),
                            )
                            ind = ipool.tile([128, hmax, WIN], BF, tag="ind",
                                             bufs=6, name=f"in{layer}{g}{w}")
                            nc.vector.tensor_tensor(
                                out=ind[:, 0:hn, :],
                                in0=dl_t[:, ho:ho + hn].to_broadcast(
                                    [128, hn, WIN]),
                                in1=iota_t[:, None, :].to_broadcast(
                                    [128, hn, WIN]),
                                op=mybir.AluOpType.is_equal,
                            )
                            parts.append((gb, ind))
                        i = 0
                        for tt in range(NSEG):
                            nk = int(kq[g, w, tt])
                            f0 = f0s[(g, w, tt)]
                            for j in range(nk):
                                fo = int(f0[j])
                                stop = (w == NW - 1 and j == nk - 1)
                                q = min(np.searchsorted(
                                    bnds, o + i, side="right") - 1, SUB - 1)
                                gb, ind = parts[q]
                                i2 = o + i - bnds[q]
                                if layer == 1:
                                    nc.tensor.matmul(
                                        out=aggs[tt][:, fo:fo + WIN],
                                        lhsT=gb[:, i2, :],
                                        rhs=ind[:, i2, :],
                                        start=False, stop=stop,
                                        skip_group_check=True)
                                else:
                                    nc.tensor.matmul(
                                        out=aggs[tt][0:CPAD, fo:fo + WIN],
                                        lhsT=gb[:, i2, 0:CPAD],
                                        rhs=ind[:, i2, :],
                                        start=False, stop=stop,
                                        skip_group_check=True)
                                i += 1
                    # finalize blocks
                    for r in range(GB):
                        b = g * GB + r
                        tt, fo = r // 4, (r % 4) * 128
                        if layer == 1:
                            t1 = fpool.tile([128, 128], FP, tag="t1")
                            nc.vector.tensor_tensor(
                                out=t1[:, :], in0=aggs[tt][:, fo:fo + 128],
                                in1=drt_g[:, r * 128:(r + 1) * 128],
                                op=mybir.AluOpType.mult)
                            r1 = fpool.tile([128, 128], BF, tag="r1")
                            nc.scalar.activation(out=r1[:, :], in_=t1[:, :],
                                                 func=AF.Relu,
                                                 bias=b1_t[:, :1])
                            ps2 = pspool.tile([128, CPAD], FP, tag="psml",
                                              bufs=2)
                            nc.tensor.matmul(out=ps2[:, :], lhsT=r1[:, :],
                                             rhs=w2_t[:, :],
                                             start=True, stop=True)
                            nc.scalar.activation(out=h2p_sb[:, b, :],
                                                 in_=ps2[:, :],
                                                 func=AF.Copy,
                                                 scale=dpc_t[:, b:b + 1])
                        else:
                            t3 = fpool.tile([CPAD, 128], FP, tag="t3")
                            nc.vector.tensor_tensor(
                                out=t3[:, :],
                                in0=aggs[tt][0:CPAD, fo:fo + 128],
                                in1=drt_g[0:CPAD, r * 128:(r + 1) * 128],
                                op=mybir.AluOpType.mult)
                            nc.vector.tensor_tensor(
                                out=outg[0:CPAD, r * 128:(r + 1) * 128],
                                in0=t3[:, :], in1=b2_t[:, :],
                                op=mybir.AluOpType.add)
                    # group epilogue
                    if layer == 2:
                        nc.sync.dma_start(
                            out=out_sT[:, g * GB * 128:(g + 1) * GB * 128],
                            in_=outg[:, :])

            agg_phase(1)
            nc.sync.dma_start(out=ag2_in[:, :, 0:CPAD], in_=h2p_sb[:, :, :])
            nc.gpsimd.collective_compute(
                "AllGather", mybir.AluOpType.bypass,
                replica_groups=[list(range(W))],
                ins=[ag2_in[:, :, :]], outs=[ag2_out[:, :]],
            )
            agg_phase(2)

    nc.compile()
    return nc


# ======================================================================
# kernel() entry point
# ======================================================================
import os as _os


LAST_EXEC_NS = None
LAST_RES = None


def kernel(x, edge_index, W1, b1, W2, b2):
    """Full-input GCN kernel: shards across 8 NeuronCores internally."""
    global LAST_EXEC_NS, LAST_RES
    import numpy as _np

    trace = bool(int(_os.environ.get("GCN_TRACE", "0")))
    if trace:
        # Optional NTFF profiling shim (axon): non-fatal if unavailable.
        try:
            import sys as _sys
            import types as _types
            from trn_agent_boot.trn_boot import _ntff_profile_via_ctypes
            if "antenv.axon_hooks" not in _sys.modules:
                _hook = _ntff_profile_via_ctypes("/opt/axon/libaxon_pjrt.so")
                _m = _types.ModuleType("antenv.axon_hooks")
                _m.get_axon_ntff_profile_hook = lambda: _hook
                _m.set_axon_ntff_profile_hook = lambda h: None
                _sys.modules["antenv.axon_hooks"] = _m
        except Exception:
            trace = False

    from concourse.bass_utils import run_bass_kernel_spmd

    cfg = Cfg()
    per_core, meta, _ = preprocess(cfg, x, edge_index, W1, b1, W2, b2)
    nc = build(cfg, meta)
    res = run_bass_kernel_spmd(
        nc, per_core, core_ids=list(range(cfg.W)), trace=trace,
    )
    LAST_EXEC_NS = res.exec_time_ns
    LAST_RES = res
    outs = [res.results[c]["out_sT"] for c in range(cfg.W)]
    return _np.ascontiguousarray(postprocess(cfg, outs, meta).astype(_np.float32))
